# revision 12
# baseline (speedup 1.0000x reference)
"""Trainium2 Bass kernel for nn_CustomEncoderLayer (dense transformer encoder layer).

Sharding: pure data-parallel over batch — 8 batch elements -> 8 NeuronCores,
each core runs the full encoder layer on its [1024, 1024] slice. Weights are
replicated to every core; no collectives.

Per-core pipeline (S=1024 tokens, D=1024, H=16 heads, Dh=64, F=4096):
  A. load src (token-major, f32), LayerNorm1 (gamma/beta folded into W on host),
     PE-transpose x -> feature-major bf16
  B. Q^T,K^T projections (feature-major, bf16, quad-blocked weight streaming),
     V (token-major, interleaved with a ones-column per head that accumulates
     the softmax denominator during the ctx matmul)
  C. attention, software-pipelined across heads: head h's scoresT+exp overlap
     head h-1's ctx matmuls. scoresT = k_h^T q_h (PSUM f32) -> exp (ACT,
     scale=1/8; no max subtraction — |scores| < ~3 so exp is safe in f32) ->
     ctx_augT = [v_h|1]^T exp^T (row 64 = denominator) -> normalize via
     reciprocal_approx_fast + gpsimd partition-broadcast + DVE multiply
  D. attn_out = ctx^T-stationary matmul with Wo, + residual (f32, in-place)
  E. LayerNorm2, transpose -> y feature-major bf16
  F. FFN: mid = relu(y@W1+b1) (bf16, 8MB resident, quad-blocked W1 streaming),
     out = mid@W2 + b2 + residual (W2 streamed in two s-halves)

All matmul operands are bf16 (fp32 PSUM accumulation). LayerNorm stats and
residuals stay fp32. Expected rel err vs fp32 reference ~1e-3.
"""

from functools import lru_cache

import numpy as np
import ml_dtypes

import concourse.bacc as bacc
import concourse.mybir as mybir
from concourse import masks
from concourse.tile import TileContext
from concourse.bass_utils import run_bass_kernel_spmd

F32 = mybir.dt.float32
BF16 = mybir.dt.bfloat16
AF = mybir.ActivationFunctionType

B, S, D = 8, 1024, 1024
H, DH = 16, 64
FF = 4096
EPS = 1e-5
SC = S // 128   # 8 token chunks
DC = D // 128   # 8 feature chunks
FC = FF // 128  # 32 ff chunks
N2 = (0, 512)   # free-dim halves


def _mm_halves(nc, ps, lhsT, rhs_full, start, stop):
    """Two N=512 matmuls covering a [*, 1024] psum tile."""
    for n0 in N2:
        nc.tensor.matmul(ps[:, n0:n0 + 512], lhsT, rhs_full[:, n0:n0 + 512],
                         start=start, stop=stop)


def build_nc(skip_bias=(False, False, False)):
    """skip_bias = (bv_zero, bo_zero, b2_zero): skip K=1 bias matmuls."""
    bv_zero, bo_zero, b2_zero = skip_bias
    nc = bacc.Bacc("TRN2", target_bir_lowering=False, num_devices=8)

    src_d = nc.dram_tensor("src", [S, D], F32, kind="ExternalInput")
    wq_d = nc.dram_tensor("wq", [D, D], BF16, kind="ExternalInput")
    wk_d = nc.dram_tensor("wk", [D, D], BF16, kind="ExternalInput")
    wv_d = nc.dram_tensor("wv", [D, D], BF16, kind="ExternalInput")
    wo_d = nc.dram_tensor("wo", [D, D], BF16, kind="ExternalInput")
    w1_d = nc.dram_tensor("w1", [D, FF], BF16, kind="ExternalInput")
    w2_d = nc.dram_tensor("w2", [FF, D], BF16, kind="ExternalInput")
    bq_t_d = nc.dram_tensor("bq_t", [128, DC], F32, kind="ExternalInput")
    bk_t_d = nc.dram_tensor("bk_t", [128, DC], F32, kind="ExternalInput")
    bv_row_d = nc.dram_tensor("bv_row", [1, D], BF16, kind="ExternalInput")
    bo_row_d = nc.dram_tensor("bo_row", [1, D], BF16, kind="ExternalInput")
    b1_t_d = nc.dram_tensor("b1_t", [128, FC], F32, kind="ExternalInput")
    b2_row_d = nc.dram_tensor("b2_row", [1, D], BF16, kind="ExternalInput")
    out_d = nc.dram_tensor("out", [S, D], F32, kind="ExternalOutput")

    with TileContext(nc) as tc:
        # ---- pools (stack allocator: release order = reverse alloc order) ----
        persist = tc.alloc_tile_pool(name="persist", bufs=1)
        ps_a = tc.alloc_tile_pool(name="ps_a", bufs=2, space="PSUM")
        ps_b = tc.alloc_tile_pool(name="ps_b", bufs=2, space="PSUM")
        p_wst = tc.alloc_tile_pool(name="p_wst", bufs=4)  # streamed weight blocks
        p_ln = tc.alloc_tile_pool(name="p_ln", bufs=4)    # LN stats scratch
        p_src = tc.alloc_tile_pool(name="p_src", bufs=1)
        p_qkv = tc.alloc_tile_pool(name="p_qkv", bufs=1)
        p_ln1 = tc.alloc_tile_pool(name="p_ln1", bufs=1)

        # ---- persistent small tiles ----
        ident = persist.tile([128, 128], F32, tag="ident")
        masks.make_identity(nc, ident[:])
        eps_t = persist.tile([128, 1], F32, tag="eps")
        nc.vector.memset(eps_t[:], EPS)
        ones_bf = persist.tile([1, 128], BF16, tag="ones")
        nc.vector.memset(ones_bf[:], 1.0)
        bq_t = persist.tile([128, DC], F32, tag="bq")
        nc.sync.dma_start(bq_t[:], bq_t_d[:])
        bk_t = persist.tile([128, DC], F32, tag="bk")
        nc.sync.dma_start(bk_t[:], bk_t_d[:])
        bv_row = persist.tile([1, D], BF16, tag="bv")
        nc.sync.dma_start(bv_row[:], bv_row_d[:])
        bo_row = persist.tile([1, D], BF16, tag="bo")
        nc.sync.dma_start(bo_row[:], bo_row_d[:])
        b1_t = persist.tile([128, FC], F32, tag="b1")
        nc.sync.dma_start(b1_t[:], b1_t_d[:])
        b2_row = persist.tile([1, D], BF16, tag="b2")
        nc.sync.dma_start(b2_row[:], b2_row_d[:])

        def quad_psums(label):
            ps4 = [ps_a.tile([128, S], F32, tag="big", name=f"{label}a{i}")
                   for i in range(2)]
            ps4 += [ps_b.tile([128, S], F32, tag="ctx", name=f"{label}b{i}")
                    for i in range(2)]
            return ps4

        def layernorm(src_tiles, out_pool, out_tag):
            """f32 token-major tiles -> normalized f32 token-major tiles."""
            normed = []
            for i in range(SC):
                st = src_tiles[i]
                stats = p_ln.tile([128, 2, 6], F32, tag="stats",
                                  name=f"st_{out_tag}{i}")
                nc.vector.bn_stats(stats[:, 0, :], st[:, 0:512])
                nc.vector.bn_stats(stats[:, 1, :], st[:, 512:1024])
                mv = p_ln.tile([128, 2], F32, tag="mv", name=f"mv_{out_tag}{i}")
                nc.vector.bn_aggr(mv[:], stats[:])
                nc.scalar.activation(mv[:, 1:2], mv[:, 1:2], AF.Sqrt,
                                     bias=eps_t[:, 0:1])
                rsig = p_ln.tile([128, 1], F32, tag="rsig",
                                 name=f"rs_{out_tag}{i}")
                nc.vector.reciprocal_approx_fast(rsig[:], mv[:, 1:2])
                xt = out_pool.tile([128, D], F32, tag=f"{out_tag}{i}",
                                   name=f"x_{out_tag}{i}")
                nc.vector.tensor_scalar(
                    out=xt[:], in0=st[:], scalar1=mv[:, 0:1], scalar2=rsig[:],
                    op0=mybir.AluOpType.subtract, op1=mybir.AluOpType.mult)
                normed.append(xt)
            return normed

        def transpose_to_fm(x_tm_tiles, out_pool, out_tag):
            """f32 token-major [128,1024]x8 -> bf16 feature-major [128,1024]x8."""
            fm = []
            for j in range(DC):
                pt = ps_a.tile([128, S], F32, tag="big", name=f"pt_{out_tag}{j}")
                for i in range(SC):
                    nc.tensor.transpose(pt[:, i * 128:(i + 1) * 128],
                                        x_tm_tiles[i][:, j * 128:(j + 1) * 128],
                                        ident[:])
                ft = out_pool.tile([128, S], BF16, tag=f"{out_tag}{j}",
                                   name=f"f_{out_tag}{j}")
                nc.scalar.copy(ft[:], pt[:])
                fm.append(ft)
            return fm

        # ================= Phase A: load + LN1 + transpose =================
        src_t = []
        for i in range(SC):
            st = p_src.tile([128, D], F32, tag=f"src{i}", name=f"src{i}")
            nc.sync.dma_start(st[:], src_d[i * 128:(i + 1) * 128, :])
            src_t.append(st)

        x_tm = layernorm(src_t, p_ln1, "xtm")
        x_fm = transpose_to_fm(x_tm, p_ln1, "xfm")

        # ================= Phase B: QKV projections =================
        wv_sb = []
        for k in range(DC):
            wt = p_qkv.tile([128, D], BF16, tag=f"wv{k}", name=f"wv{k}")
            nc.sync.dma_start(wt[:], wv_d[k * 128:(k + 1) * 128, :])
            wv_sb.append(wt)

        qt_t, kt_t = [], []
        for nm, w_d, b_t, dst in (("q", wq_d, bq_t, qt_t), ("k", wk_d, bk_t, kt_t)):
            for mq in range(2):
                ps4 = quad_psums(f"{nm}{mq}")
                for k in range(DC):
                    wblk = p_wst.tile([128, 512], BF16, tag="wblk",
                                      name=f"w{nm}{mq}_{k}")
                    nc.sync.dma_start(
                        wblk[:], w_d[k * 128:(k + 1) * 128, mq * 512:(mq + 1) * 512])
                    for i in range(4):
                        _mm_halves(nc, ps4[i], wblk[:, i * 128:(i + 1) * 128],
                                   x_fm[k][:], start=(k == 0), stop=(k == DC - 1))
                for i in range(4):
                    m = mq * 4 + i
                    ot = p_qkv.tile([128, S], BF16, tag=f"{nm}t{m}",
                                    name=f"{nm}t{m}")
                    nc.scalar.activation(ot[:], ps4[i][:], AF.Identity,
                                         bias=b_t[:, m:m + 1])
                    dst.append(ot)

        # V token-major, interleaved [128, 16, 65] with ones column at 64
        v_aug = []
        for s in range(SC):
            ps = ps_a.tile([128, S], F32, tag="big", name=f"vps{s}")
            for k in range(DC):
                _mm_halves(nc, ps, x_fm[k][:, s * 128:(s + 1) * 128], wv_sb[k][:],
                           start=(k == 0), stop=(k == DC - 1 and bv_zero))
            if not bv_zero:
                for n0 in N2:
                    nc.tensor.matmul(ps[:, n0:n0 + 512], ones_bf[0:1, 0:128],
                                     bv_row[0:1, n0:n0 + 512], start=False,
                                     stop=True)
            va = p_qkv.tile([128, H, DH + 1], BF16, tag=f"va{s}", name=f"va{s}")
            nc.vector.memset(va[:, :, DH:DH + 1], 1.0)
            nc.scalar.copy(va[:, :, 0:DH],
                           ps[:].rearrange("p (h c) -> p h c", c=DH))
            v_aug.append(va)

        p_ln1.release()  # x_tm / x_fm no longer needed

        # ================= Phase C: attention (head-pipelined) =================
        p_wo = tc.alloc_tile_pool(name="p_wo", bufs=1)
        p_ctx = tc.alloc_tile_pool(name="p_ctx", bufs=1)
        p_attn = tc.alloc_tile_pool(name="p_attn", bufs=2)

        wo_sb = []
        for k in range(DC):
            wt = p_wo.tile([128, D], BF16, tag=f"wo{k}", name=f"wo{k}")
            nc.sync.dma_start(wt[:], wo_d[k * 128:(k + 1) * 128, :])
            wo_sb.append(wt)

        ctx_t = [p_ctx.tile([128, S], BF16, tag=f"ctx{t}", name=f"ctx{t}")
                 for t in range(DC)]

        prev = None  # (head, exp_tiles) from previous iteration
        for h in range(H + 1):
            if h < H:
                t, off = h // 2, (h % 2) * 64
                q_sl = qt_t[t][off:off + 64, :]
                exp_tiles = []
            psc = None
            if prev is not None:
                ph = prev[0]
                psc = ps_b.tile([DH + 1, S], F32, tag="ctx", name=f"psc{ph}")
            # blocked sub-sequences: all scores of head h, then all ctx of h-1
            # (ctx never waits on exp: one full head of slack; one PE config
            # switch per block instead of per matmul pair)
            if h < H:
                for sk in range(SC):
                    k_sl = kt_t[t][off:off + 64, sk * 128:(sk + 1) * 128]
                    pss = ps_a.tile([128, S], F32, tag="big", name=f"pss{h}_{sk}")
                    _mm_halves(nc, pss, k_sl, q_sl, start=True, stop=True)
                    et = p_attn.tile([128, S], BF16, tag=f"e{sk}",
                                     name=f"e{h}_{sk}")
                    nc.scalar.activation(et[:], pss[:], AF.Exp, scale=1.0 / 8.0)
                    exp_tiles.append(et)
            if prev is not None:
                ph, pexp = prev
                for sk in range(SC):
                    _mm_halves(nc, psc, v_aug[sk][:, ph, :], pexp[sk][:],
                               start=(sk == 0), stop=(sk == SC - 1))
            if prev is not None:
                ph = prev[0]
                pt_, poff = ph // 2, (ph % 2) * 64
                den = p_attn.tile([1, S], F32, tag="den", name=f"den{ph}")
                nc.scalar.copy(den[:], psc[DH:DH + 1, :])
                rec = p_attn.tile([1, S], F32, tag="recip", name=f"rec{ph}")
                nc.vector.reciprocal_approx_fast(rec[:], den[:])
                rec_bf = p_attn.tile([1, S], BF16, tag="recbf", name=f"rb{ph}")
                nc.vector.tensor_copy(rec_bf[:], rec[:])
                # broadcast via K=1 matmul (PE is underutilized here), then
                # ACT copy to SBUF for the DVE multiply
                psr = ps_a.tile([128, S], F32, tag="big", name=f"psr{ph}")
                for n0 in N2:
                    nc.tensor.matmul(psr[0:64, n0:n0 + 512], ones_bf[0:1, 0:64],
                                     rec_bf[0:1, n0:n0 + 512],
                                     start=True, stop=True)
                bc = p_attn.tile([64, S], F32, tag="bcast", name=f"bc{ph}")
                nc.scalar.copy(bc[:], psr[0:64, :])
                nc.vector.tensor_tensor(
                    out=ctx_t[pt_][poff:poff + 64, :], in0=psc[0:DH, :],
                    in1=bc[:], op=mybir.AluOpType.mult)
            prev = (h, exp_tiles) if h < H else None

        # ================= Phase D: Wo projection + residual (in-place) ========
        for s in range(SC):
            ps = ps_a.tile([128, S], F32, tag="big", name=f"ops{s}")
            for k in range(DC):
                _mm_halves(nc, ps, ctx_t[k][:, s * 128:(s + 1) * 128], wo_sb[k][:],
                           start=(k == 0), stop=(k == DC - 1 and bo_zero))
            if not bo_zero:
                for n0 in N2:
                    nc.tensor.matmul(ps[:, n0:n0 + 512], ones_bf[0:1, 0:128],
                                     bo_row[0:1, n0:n0 + 512], start=False,
                                     stop=True)
            nc.vector.tensor_add(src_t[s][:], ps[:], src_t[s][:])
        src2_t = src_t

        p_attn.release()
        p_ctx.release()
        p_wo.release()
        p_qkv.release()

        # ================= Phase E: LN2 + transpose =================
        p_ffn = tc.alloc_tile_pool(name="p_ffn", bufs=1)
        p_out = tc.alloc_tile_pool(name="p_out", bufs=3)
        p_ytm = tc.alloc_tile_pool(name="p_ytm", bufs=1)

        y_tm = layernorm(src2_t, p_ytm, "ytm")
        y_fm = transpose_to_fm(y_tm, p_ffn, "yfm")
        p_ytm.release()

        # ================= Phase F: FFN =================
        mid_t = []
        for fq in range(FC // 4):
            ps4 = quad_psums(f"m{fq}")
            for k in range(DC):
                w1q = p_wst.tile([128, 512], BF16, tag="wblk", name=f"w1_{fq}_{k}")
                nc.sync.dma_start(
                    w1q[:], w1_d[k * 128:(k + 1) * 128, fq * 512:(fq + 1) * 512])
                for i in range(4):
                    _mm_halves(nc, ps4[i], w1q[:, i * 128:(i + 1) * 128],
                               y_fm[k][:], start=(k == 0), stop=(k == DC - 1))
            for i in range(4):
                f = fq * 4 + i
                mt = p_ffn.tile([128, S], BF16, tag=f"mid{f}", name=f"mid{f}")
                nc.scalar.activation(mt[:], ps4[i][:], AF.Relu,
                                     bias=b1_t[:, f:f + 1])
                mid_t.append(mt)

        for half in range(2):
            ps4 = quad_psums(f"o{half}")
            for k in range(FC):
                w2t = p_wst.tile([128, D], BF16, tag="w2s", name=f"w2_{half}_{k}")
                nc.sync.dma_start(w2t[:], w2_d[k * 128:(k + 1) * 128, :])
                for i in range(4):
                    s = half * 4 + i
                    _mm_halves(nc, ps4[i],
                               mid_t[k][:, s * 128:(s + 1) * 128], w2t[:],
                               start=(k == 0), stop=(k == FC - 1 and b2_zero))
            for i in range(4):
                s = half * 4 + i
                ps = ps4[i]
                if not b2_zero:
                    for n0 in N2:
                        nc.tensor.matmul(ps[:, n0:n0 + 512], ones_bf[0:1, 0:128],
                                         b2_row[0:1, n0:n0 + 512],
                                         start=False, stop=True,
                                         skip_group_check=True)
                ot = p_out.tile([128, D], F32, tag="outbuf", name=f"ob{s}")
                nc.vector.tensor_add(ot[:], ps[:], src2_t[s][:])
                nc.sync.dma_start(out_d[s * 128:(s + 1) * 128, :], ot[:])

        p_out.release()
        p_ffn.release()
        p_src.release()
        p_ln.release()
        p_wst.release()
        ps_b.release()
        ps_a.release()
        persist.release()

    nc.finalize()
    return nc


@lru_cache(maxsize=2)
def _get_nc(skip_bias):
    return build_nc(skip_bias)


def _prep_weights(inputs):
    bf = ml_dtypes.bfloat16
    f32 = np.float32
    g1 = np.asarray(inputs["g1"], f32)
    beta1 = np.asarray(inputs["beta1"], f32)
    g2 = np.asarray(inputs["g2"], f32)
    beta2 = np.asarray(inputs["beta2"], f32)
    Wq = np.asarray(inputs["Wq"], f32)
    Wk = np.asarray(inputs["Wk"], f32)
    Wv = np.asarray(inputs["Wv"], f32)
    Wo = np.asarray(inputs["Wo"], f32)
    W1 = np.asarray(inputs["W1"], f32)
    W2 = np.asarray(inputs["W2"], f32)

    # fold LN1 gamma/beta into QKV weights+biases, LN2 into W1/b1
    wq_eff = g1[:, None] * Wq
    wk_eff = g1[:, None] * Wk
    wv_eff = g1[:, None] * Wv
    bq_eff = np.asarray(inputs["bq"], f32) + beta1 @ Wq
    bk_eff = np.asarray(inputs["bk"], f32) + beta1 @ Wk
    bv_eff = np.asarray(inputs["bv"], f32) + beta1 @ Wv
    w1_eff = g2[:, None] * W1
    b1_eff = np.asarray(inputs["b1"], f32) + beta2 @ W1
    b2_eff = np.asarray(inputs["b2"], f32)
    bo_eff = np.asarray(inputs["bo"], f32)

    weights = {
        "wq": np.ascontiguousarray(wq_eff.astype(bf)),
        "wk": np.ascontiguousarray(wk_eff.astype(bf)),
        "wv": np.ascontiguousarray(wv_eff.astype(bf)),
        "wo": np.ascontiguousarray(Wo.astype(bf)),
        "w1": np.ascontiguousarray(w1_eff.astype(bf)),
        "w2": np.ascontiguousarray(W2.astype(bf)),
        "bq_t": np.ascontiguousarray(bq_eff.reshape(DC, 128).T.astype(f32)),
        "bk_t": np.ascontiguousarray(bk_eff.reshape(DC, 128).T.astype(f32)),
        "bv_row": np.ascontiguousarray(bv_eff.reshape(1, D).astype(bf)),
        "bo_row": np.ascontiguousarray(bo_eff.reshape(1, D).astype(bf)),
        "b1_t": np.ascontiguousarray(b1_eff.reshape(FC, 128).T.astype(f32)),
        "b2_row": np.ascontiguousarray(b2_eff.reshape(1, D).astype(bf)),
    }
    skip_bias = (bool(np.all(bv_eff == 0)), bool(np.all(bo_eff == 0)),
                 bool(np.all(b2_eff == 0)))
    return weights, skip_bias


def run_kernel(inputs, trace=False):
    weights, skip_bias = _prep_weights(inputs)
    nc = _get_nc(skip_bias)
    src = np.asarray(inputs["src"], np.float32)
    in_maps = [dict(weights, src=np.ascontiguousarray(src[i])) for i in range(B)]
    res = run_bass_kernel_spmd(nc, in_maps, list(range(B)), trace=trace)
    out = np.stack([res.results[i]["out"] for i in range(B)], axis=0)
    return out, res


def kernel(**inputs) -> np.ndarray:
    out, _ = run_kernel(inputs, trace=False)
    return out


# revision 13
# speedup vs baseline: 1.0888x; 1.0888x over previous
"""Trainium2 Bass kernel for nn_CustomEncoderLayer (dense transformer encoder layer).

Sharding: pure data-parallel over batch — 8 batch elements -> 8 NeuronCores,
each core runs the full encoder layer on its [1024, 1024] slice. Weights are
replicated to every core; no collectives.

Per-core pipeline (S=1024 tokens, D=1024, H=16 heads, Dh=64, F=4096):
  A. load src (token-major, f32), LayerNorm1 (gamma/beta folded into W on host),
     PE-transpose x -> feature-major bf16
  B. Q^T,K^T projections (feature-major, bf16, quad-blocked weight streaming),
     V (token-major, interleaved with a ones-column per head that accumulates
     the softmax denominator during the ctx matmul)
  C. attention, software-pipelined across heads: head h's scoresT+exp overlap
     head h-1's ctx matmuls. scoresT = k_h^T q_h (PSUM f32) -> exp (ACT,
     scale=1/8; no max subtraction — |scores| < ~3 so exp is safe in f32) ->
     ctx_augT = [v_h|1]^T exp^T (row 64 = denominator) -> normalize via
     reciprocal_approx_fast + gpsimd partition-broadcast + DVE multiply
  D. attn_out = ctx^T-stationary matmul with Wo, + residual (f32, in-place)
  E. LayerNorm2, transpose -> y feature-major bf16
  F. FFN: mid = relu(y@W1+b1) (bf16, 8MB resident, quad-blocked W1 streaming),
     out = mid@W2 + b2 + residual (W2 streamed in two s-halves)

All matmul operands are bf16 (fp32 PSUM accumulation). LayerNorm stats and
residuals stay fp32. Expected rel err vs fp32 reference ~1e-3.
"""

from functools import lru_cache

import numpy as np
import ml_dtypes

import concourse.bacc as bacc
import concourse.mybir as mybir
from concourse import masks
from concourse.tile import TileContext
from concourse.bass_utils import run_bass_kernel_spmd

F32 = mybir.dt.float32
BF16 = mybir.dt.bfloat16
AF = mybir.ActivationFunctionType

B, S, D = 8, 1024, 1024
H, DH = 16, 64
FF = 4096
EPS = 1e-5
SC = S // 128   # 8 token chunks
DC = D // 128   # 8 feature chunks
FC = FF // 128  # 32 ff chunks
N2 = (0, 512)   # free-dim halves


def _mm_halves(nc, ps, lhsT, rhs_full, start, stop):
    """Two N=512 matmuls covering a [*, 1024] psum tile."""
    for n0 in N2:
        nc.tensor.matmul(ps[:, n0:n0 + 512], lhsT, rhs_full[:, n0:n0 + 512],
                         start=start, stop=stop)


def build_nc(skip_bias=(False, False, False)):
    """skip_bias = (bv_zero, bo_zero, b2_zero): skip K=1 bias matmuls."""
    bv_zero, bo_zero, b2_zero = skip_bias
    nc = bacc.Bacc("TRN2", target_bir_lowering=False, num_devices=8)

    src_d = nc.dram_tensor("src", [S, D], F32, kind="ExternalInput")
    wq_d = nc.dram_tensor("wq", [D, D], BF16, kind="ExternalInput")
    wk_d = nc.dram_tensor("wk", [D, D], BF16, kind="ExternalInput")
    wv_d = nc.dram_tensor("wv", [D, D], BF16, kind="ExternalInput")
    wo_d = nc.dram_tensor("wo", [D, D], BF16, kind="ExternalInput")
    w1_d = nc.dram_tensor("w1", [D, FF], BF16, kind="ExternalInput")
    w2_d = nc.dram_tensor("w2", [FF, D], BF16, kind="ExternalInput")
    bq_t_d = nc.dram_tensor("bq_t", [128, DC], F32, kind="ExternalInput")
    bk_t_d = nc.dram_tensor("bk_t", [128, DC], F32, kind="ExternalInput")
    bv_row_d = nc.dram_tensor("bv_row", [1, D], BF16, kind="ExternalInput")
    bo_row_d = nc.dram_tensor("bo_row", [1, D], BF16, kind="ExternalInput")
    b1_t_d = nc.dram_tensor("b1_t", [128, FC], F32, kind="ExternalInput")
    b2_row_d = nc.dram_tensor("b2_row", [1, D], BF16, kind="ExternalInput")
    out_d = nc.dram_tensor("out", [S, D], F32, kind="ExternalOutput")

    with TileContext(nc) as tc:
        # ---- pools (stack allocator: release order = reverse alloc order) ----
        persist = tc.alloc_tile_pool(name="persist", bufs=1)
        ps_a = tc.alloc_tile_pool(name="ps_a", bufs=2, space="PSUM")
        ps_b = tc.alloc_tile_pool(name="ps_b", bufs=2, space="PSUM")
        p_wst = tc.alloc_tile_pool(name="p_wst", bufs=4)  # streamed weight blocks
        p_ln = tc.alloc_tile_pool(name="p_ln", bufs=4)    # LN stats scratch
        p_src = tc.alloc_tile_pool(name="p_src", bufs=1)
        p_qkv = tc.alloc_tile_pool(name="p_qkv", bufs=1)
        p_ln1 = tc.alloc_tile_pool(name="p_ln1", bufs=1)

        # ---- persistent small tiles ----
        ident = persist.tile([128, 128], F32, tag="ident")
        masks.make_identity(nc, ident[:])
        eps_t = persist.tile([128, 1], F32, tag="eps")
        nc.vector.memset(eps_t[:], EPS)
        ones_bf = persist.tile([1, 128], BF16, tag="ones")
        nc.vector.memset(ones_bf[:], 1.0)
        bq_t = persist.tile([128, DC], F32, tag="bq")
        nc.sync.dma_start(bq_t[:], bq_t_d[:])
        bk_t = persist.tile([128, DC], F32, tag="bk")
        nc.sync.dma_start(bk_t[:], bk_t_d[:])
        bv_row = persist.tile([1, D], BF16, tag="bv")
        nc.sync.dma_start(bv_row[:], bv_row_d[:])
        bo_row = persist.tile([1, D], BF16, tag="bo")
        nc.sync.dma_start(bo_row[:], bo_row_d[:])
        b1_t = persist.tile([128, FC], F32, tag="b1")
        nc.sync.dma_start(b1_t[:], b1_t_d[:])
        b2_row = persist.tile([1, D], BF16, tag="b2")
        nc.sync.dma_start(b2_row[:], b2_row_d[:])

        def quad_psums(label):
            ps4 = [ps_a.tile([128, S], F32, tag="big", name=f"{label}a{i}")
                   for i in range(2)]
            ps4 += [ps_b.tile([128, S], F32, tag="ctx", name=f"{label}b{i}")
                    for i in range(2)]
            return ps4

        def layernorm(src_tiles, out_pool, out_tag):
            """f32 token-major tiles -> normalized f32 token-major tiles."""
            normed = []
            for i in range(SC):
                st = src_tiles[i]
                stats = p_ln.tile([128, 2, 6], F32, tag="stats",
                                  name=f"st_{out_tag}{i}")
                nc.vector.bn_stats(stats[:, 0, :], st[:, 0:512])
                nc.vector.bn_stats(stats[:, 1, :], st[:, 512:1024])
                mv = p_ln.tile([128, 2], F32, tag="mv", name=f"mv_{out_tag}{i}")
                nc.vector.bn_aggr(mv[:], stats[:])
                nc.scalar.activation(mv[:, 1:2], mv[:, 1:2], AF.Sqrt,
                                     bias=eps_t[:, 0:1])
                rsig = p_ln.tile([128, 1], F32, tag="rsig",
                                 name=f"rs_{out_tag}{i}")
                nc.vector.reciprocal_approx_fast(rsig[:], mv[:, 1:2])
                xt = out_pool.tile([128, D], F32, tag=f"{out_tag}{i}",
                                   name=f"x_{out_tag}{i}")
                nc.vector.tensor_scalar(
                    out=xt[:], in0=st[:], scalar1=mv[:, 0:1], scalar2=rsig[:],
                    op0=mybir.AluOpType.subtract, op1=mybir.AluOpType.mult)
                normed.append(xt)
            return normed

        def transpose_to_fm(x_tm_tiles, out_pool, out_tag):
            """f32 token-major [128,1024]x8 -> bf16 feature-major [128,1024]x8."""
            fm = []
            for j in range(DC):
                pt = ps_a.tile([128, S], F32, tag="big", name=f"pt_{out_tag}{j}")
                for i in range(SC):
                    nc.tensor.transpose(pt[:, i * 128:(i + 1) * 128],
                                        x_tm_tiles[i][:, j * 128:(j + 1) * 128],
                                        ident[:])
                ft = out_pool.tile([128, S], BF16, tag=f"{out_tag}{j}",
                                   name=f"f_{out_tag}{j}")
                nc.scalar.copy(ft[:], pt[:])
                fm.append(ft)
            return fm

        # ================= Phase A: load + LN1 + transpose =================
        src_t = []
        for i in range(SC):
            st = p_src.tile([128, D], F32, tag=f"src{i}", name=f"src{i}")
            nc.sync.dma_start(st[:], src_d[i * 128:(i + 1) * 128, :])
            src_t.append(st)

        x_tm = layernorm(src_t, p_ln1, "xtm")
        x_fm = transpose_to_fm(x_tm, p_ln1, "xfm")

        # ================= Phase B: QKV projections =================
        wv_sb = []
        for k in range(DC):
            wt = p_qkv.tile([128, D], BF16, tag=f"wv{k}", name=f"wv{k}")
            nc.sync.dma_start(wt[:], wv_d[k * 128:(k + 1) * 128, :])
            wv_sb.append(wt)

        qt_t, kt_t = [], []
        for nm, w_d, b_t, dst in (("q", wq_d, bq_t, qt_t), ("k", wk_d, bk_t, kt_t)):
            for mq in range(2):
                ps4 = quad_psums(f"{nm}{mq}")
                for k in range(DC):
                    wblk = p_wst.tile([128, 512], BF16, tag="wblk",
                                      name=f"w{nm}{mq}_{k}")
                    nc.sync.dma_start(
                        wblk[:], w_d[k * 128:(k + 1) * 128, mq * 512:(mq + 1) * 512])
                    for i in range(4):
                        _mm_halves(nc, ps4[i], wblk[:, i * 128:(i + 1) * 128],
                                   x_fm[k][:], start=(k == 0), stop=(k == DC - 1))
                for i in range(4):
                    m = mq * 4 + i
                    ot = p_qkv.tile([128, S], BF16, tag=f"{nm}t{m}",
                                    name=f"{nm}t{m}")
                    nc.scalar.activation(ot[:], ps4[i][:], AF.Identity,
                                         bias=b_t[:, m:m + 1])
                    dst.append(ot)

        # V token-major, interleaved [128, 16, 65] with ones column at 64
        v_aug = []
        for s in range(SC):
            ps = ps_a.tile([128, S], F32, tag="big", name=f"vps{s}")
            for k in range(DC):
                _mm_halves(nc, ps, x_fm[k][:, s * 128:(s + 1) * 128], wv_sb[k][:],
                           start=(k == 0), stop=(k == DC - 1 and bv_zero))
            if not bv_zero:
                for n0 in N2:
                    nc.tensor.matmul(ps[:, n0:n0 + 512], ones_bf[0:1, 0:128],
                                     bv_row[0:1, n0:n0 + 512], start=False,
                                     stop=True)
            va = p_qkv.tile([128, H, DH + 1], BF16, tag=f"va{s}", name=f"va{s}")
            nc.vector.memset(va[:, :, DH:DH + 1], 1.0)
            nc.scalar.copy(va[:, :, 0:DH],
                           ps[:].rearrange("p (h c) -> p h c", c=DH))
            v_aug.append(va)

        p_ln1.release()  # x_tm / x_fm no longer needed

        # ================= Phase C: attention (head-pipelined) =================
        p_wo = tc.alloc_tile_pool(name="p_wo", bufs=1)
        p_ctx = tc.alloc_tile_pool(name="p_ctx", bufs=1)
        p_attn = tc.alloc_tile_pool(name="p_attn", bufs=2)

        wo_sb = []
        for k in range(DC):
            wt = p_wo.tile([128, D], BF16, tag=f"wo{k}", name=f"wo{k}")
            nc.sync.dma_start(wt[:], wo_d[k * 128:(k + 1) * 128, :])
            wo_sb.append(wt)

        ctx_t = [p_ctx.tile([128, S], BF16, tag=f"ctx{t}", name=f"ctx{t}")
                 for t in range(DC)]

        prev = None  # (head, exp_tiles) from previous iteration
        for h in range(H + 1):
            if h < H:
                t, off = h // 2, (h % 2) * 64
                q_sl = qt_t[t][off:off + 64, :]
                exp_tiles = []
            psc = None
            if prev is not None:
                ph = prev[0]
                psc = ps_b.tile([DH + 1, S], F32, tag="ctx", name=f"psc{ph}")
            # blocked sub-sequences: all scores of head h, then all ctx of h-1
            # (ctx never waits on exp: one full head of slack; one PE config
            # switch per block instead of per matmul pair)
            if h < H:
                for sk in range(SC):
                    k_sl = kt_t[t][off:off + 64, sk * 128:(sk + 1) * 128]
                    pss = ps_a.tile([128, S], F32, tag="big", name=f"pss{h}_{sk}")
                    _mm_halves(nc, pss, k_sl, q_sl, start=True, stop=True)
                    et = p_attn.tile([128, S], BF16, tag=f"e{sk}",
                                     name=f"e{h}_{sk}")
                    nc.scalar.activation(et[:], pss[:], AF.Exp, scale=1.0 / 8.0)
                    exp_tiles.append(et)
            if prev is not None:
                ph, pexp = prev
                for sk in range(SC):
                    _mm_halves(nc, psc, v_aug[sk][:, ph, :], pexp[sk][:],
                               start=(sk == 0), stop=(sk == SC - 1))
            if prev is not None:
                ph = prev[0]
                pt_, poff = ph // 2, (ph % 2) * 64
                den = p_attn.tile([1, S], F32, tag="den", name=f"den{ph}")
                nc.scalar.copy(den[:], psc[DH:DH + 1, :])
                rec = p_attn.tile([1, S], F32, tag="recip", name=f"rec{ph}")
                nc.vector.reciprocal_approx_fast(rec[:], den[:])
                bc = p_attn.tile([64, S], F32, tag="bcast", name=f"bc{ph}")
                nc.gpsimd.partition_broadcast(bc[:], rec[0:1, :])
                nc.vector.tensor_tensor(
                    out=ctx_t[pt_][poff:poff + 64, :], in0=psc[0:DH, :],
                    in1=bc[:], op=mybir.AluOpType.mult)
            prev = (h, exp_tiles) if h < H else None

        # ================= Phase D: Wo projection + residual (in-place) ========
        for s in range(SC):
            ps = ps_a.tile([128, S], F32, tag="big", name=f"ops{s}")
            for k in range(DC):
                _mm_halves(nc, ps, ctx_t[k][:, s * 128:(s + 1) * 128], wo_sb[k][:],
                           start=(k == 0), stop=(k == DC - 1 and bo_zero))
            if not bo_zero:
                for n0 in N2:
                    nc.tensor.matmul(ps[:, n0:n0 + 512], ones_bf[0:1, 0:128],
                                     bo_row[0:1, n0:n0 + 512], start=False,
                                     stop=True)
            nc.vector.tensor_add(src_t[s][:], ps[:], src_t[s][:])
        src2_t = src_t

        p_attn.release()
        p_ctx.release()
        p_wo.release()
        p_qkv.release()

        # ================= Phase E: LN2 + transpose =================
        p_ffn = tc.alloc_tile_pool(name="p_ffn", bufs=1)
        p_out = tc.alloc_tile_pool(name="p_out", bufs=3)
        p_ytm = tc.alloc_tile_pool(name="p_ytm", bufs=1)

        y_tm = layernorm(src2_t, p_ytm, "ytm")
        y_fm = transpose_to_fm(y_tm, p_ffn, "yfm")
        p_ytm.release()

        # ================= Phase F: FFN =================
        mid_t = []
        for fq in range(FC // 4):
            ps4 = quad_psums(f"m{fq}")
            for k in range(DC):
                w1q = p_wst.tile([128, 512], BF16, tag="wblk", name=f"w1_{fq}_{k}")
                nc.sync.dma_start(
                    w1q[:], w1_d[k * 128:(k + 1) * 128, fq * 512:(fq + 1) * 512])
                for i in range(4):
                    _mm_halves(nc, ps4[i], w1q[:, i * 128:(i + 1) * 128],
                               y_fm[k][:], start=(k == 0), stop=(k == DC - 1))
            for i in range(4):
                f = fq * 4 + i
                mt = p_ffn.tile([128, S], BF16, tag=f"mid{f}", name=f"mid{f}")
                nc.scalar.activation(mt[:], ps4[i][:], AF.Relu,
                                     bias=b1_t[:, f:f + 1])
                mid_t.append(mt)

        for half in range(2):
            ps4 = quad_psums(f"o{half}")
            for k in range(FC):
                w2t = p_wst.tile([128, D], BF16, tag="w2s", name=f"w2_{half}_{k}")
                nc.sync.dma_start(w2t[:], w2_d[k * 128:(k + 1) * 128, :])
                for i in range(4):
                    s = half * 4 + i
                    _mm_halves(nc, ps4[i],
                               mid_t[k][:, s * 128:(s + 1) * 128], w2t[:],
                               start=(k == 0), stop=(k == FC - 1 and b2_zero))
            for i in range(4):
                s = half * 4 + i
                ps = ps4[i]
                if not b2_zero:
                    for n0 in N2:
                        nc.tensor.matmul(ps[:, n0:n0 + 512], ones_bf[0:1, 0:128],
                                         b2_row[0:1, n0:n0 + 512],
                                         start=False, stop=True,
                                         skip_group_check=True)
                ot = p_out.tile([128, D], F32, tag="outbuf", name=f"ob{s}")
                nc.vector.tensor_add(ot[:], ps[:], src2_t[s][:])
                nc.sync.dma_start(out_d[s * 128:(s + 1) * 128, :], ot[:])

        p_out.release()
        p_ffn.release()
        p_src.release()
        p_ln.release()
        p_wst.release()
        ps_b.release()
        ps_a.release()
        persist.release()

    nc.finalize()
    return nc


@lru_cache(maxsize=2)
def _get_nc(skip_bias):
    return build_nc(skip_bias)


def _prep_weights(inputs):
    bf = ml_dtypes.bfloat16
    f32 = np.float32
    g1 = np.asarray(inputs["g1"], f32)
    beta1 = np.asarray(inputs["beta1"], f32)
    g2 = np.asarray(inputs["g2"], f32)
    beta2 = np.asarray(inputs["beta2"], f32)
    Wq = np.asarray(inputs["Wq"], f32)
    Wk = np.asarray(inputs["Wk"], f32)
    Wv = np.asarray(inputs["Wv"], f32)
    Wo = np.asarray(inputs["Wo"], f32)
    W1 = np.asarray(inputs["W1"], f32)
    W2 = np.asarray(inputs["W2"], f32)

    # fold LN1 gamma/beta into QKV weights+biases, LN2 into W1/b1
    wq_eff = g1[:, None] * Wq
    wk_eff = g1[:, None] * Wk
    wv_eff = g1[:, None] * Wv
    bq_eff = np.asarray(inputs["bq"], f32) + beta1 @ Wq
    bk_eff = np.asarray(inputs["bk"], f32) + beta1 @ Wk
    bv_eff = np.asarray(inputs["bv"], f32) + beta1 @ Wv
    w1_eff = g2[:, None] * W1
    b1_eff = np.asarray(inputs["b1"], f32) + beta2 @ W1
    b2_eff = np.asarray(inputs["b2"], f32)
    bo_eff = np.asarray(inputs["bo"], f32)

    weights = {
        "wq": np.ascontiguousarray(wq_eff.astype(bf)),
        "wk": np.ascontiguousarray(wk_eff.astype(bf)),
        "wv": np.ascontiguousarray(wv_eff.astype(bf)),
        "wo": np.ascontiguousarray(Wo.astype(bf)),
        "w1": np.ascontiguousarray(w1_eff.astype(bf)),
        "w2": np.ascontiguousarray(W2.astype(bf)),
        "bq_t": np.ascontiguousarray(bq_eff.reshape(DC, 128).T.astype(f32)),
        "bk_t": np.ascontiguousarray(bk_eff.reshape(DC, 128).T.astype(f32)),
        "bv_row": np.ascontiguousarray(bv_eff.reshape(1, D).astype(bf)),
        "bo_row": np.ascontiguousarray(bo_eff.reshape(1, D).astype(bf)),
        "b1_t": np.ascontiguousarray(b1_eff.reshape(FC, 128).T.astype(f32)),
        "b2_row": np.ascontiguousarray(b2_eff.reshape(1, D).astype(bf)),
    }
    skip_bias = (bool(np.all(bv_eff == 0)), bool(np.all(bo_eff == 0)),
                 bool(np.all(b2_eff == 0)))
    return weights, skip_bias


def run_kernel(inputs, trace=False):
    weights, skip_bias = _prep_weights(inputs)
    nc = _get_nc(skip_bias)
    src = np.asarray(inputs["src"], np.float32)
    in_maps = [dict(weights, src=np.ascontiguousarray(src[i])) for i in range(B)]
    res = run_bass_kernel_spmd(nc, in_maps, list(range(B)), trace=trace)
    out = np.stack([res.results[i]["out"] for i in range(B)], axis=0)
    return out, res


def kernel(**inputs) -> np.ndarray:
    out, _ = run_kernel(inputs, trace=False)
    return out


# revision 14
# speedup vs baseline: 1.1633x; 1.0684x over previous
"""Trainium2 Bass kernel for nn_CustomEncoderLayer (dense transformer encoder layer).

Sharding: pure data-parallel over batch — 8 batch elements -> 8 NeuronCores,
each core runs the full encoder layer on its [1024, 1024] slice. Weights are
replicated to every core; no collectives.

Per-core pipeline (S=1024 tokens, D=1024, H=16 heads, Dh=64, F=4096):
  A. load src (token-major, f32), LayerNorm1 (gamma/beta folded into W on host),
     PE-transpose x -> feature-major bf16
  B. Q^T,K^T projections (feature-major, bf16, quad-blocked weight streaming),
     V (token-major, interleaved with a ones-column per head that accumulates
     the softmax denominator during the ctx matmul)
  C. attention, software-pipelined across heads: head h's scoresT+exp overlap
     head h-1's ctx matmuls. scoresT = k_h^T q_h (PSUM f32) -> exp (ACT,
     scale=1/8; no max subtraction — |scores| < ~3 so exp is safe in f32) ->
     ctx_augT = [v_h|1]^T exp^T (row 64 = denominator) -> normalize via
     reciprocal_approx_fast + gpsimd partition-broadcast + DVE multiply
  D. attn_out = ctx^T-stationary matmul with Wo, + residual (f32, in-place)
  E. LayerNorm2, transpose -> y feature-major bf16
  F. FFN: mid = relu(y@W1+b1) (bf16, 8MB resident, quad-blocked W1 streaming),
     out = mid@W2 + b2 + residual (W2 streamed in two s-halves)

All matmul operands are bf16 (fp32 PSUM accumulation). LayerNorm stats and
residuals stay fp32. Expected rel err vs fp32 reference ~1e-3.
"""

from functools import lru_cache

import numpy as np
import ml_dtypes

import concourse.bacc as bacc
import concourse.mybir as mybir
from concourse import masks
from concourse.tile import TileContext
from concourse.bass_utils import run_bass_kernel_spmd

F32 = mybir.dt.float32
BF16 = mybir.dt.bfloat16
AF = mybir.ActivationFunctionType

B, S, D = 8, 1024, 1024
H, DH = 16, 64
FF = 4096
EPS = 1e-5
SC = S // 128   # 8 token chunks
DC = D // 128   # 8 feature chunks
FC = FF // 128  # 32 ff chunks
N2 = (0, 512)   # free-dim halves


def _mm_halves(nc, ps, lhsT, rhs_full, start, stop):
    """Two N=512 matmuls covering a [*, 1024] psum tile."""
    for n0 in N2:
        nc.tensor.matmul(ps[:, n0:n0 + 512], lhsT, rhs_full[:, n0:n0 + 512],
                         start=start, stop=stop)


def build_nc(skip_bias=(False, False, False)):
    """skip_bias = (bv_zero, bo_zero, b2_zero): skip K=1 bias matmuls."""
    bv_zero, bo_zero, b2_zero = skip_bias
    nc = bacc.Bacc("TRN2", target_bir_lowering=False, num_devices=8)

    src_d = nc.dram_tensor("src", [S, D], F32, kind="ExternalInput")
    wq_d = nc.dram_tensor("wq", [D, D], BF16, kind="ExternalInput")
    wk_d = nc.dram_tensor("wk", [D, D], BF16, kind="ExternalInput")
    wv_d = nc.dram_tensor("wv", [D, D], BF16, kind="ExternalInput")
    wo_d = nc.dram_tensor("wo", [D, D], BF16, kind="ExternalInput")
    w1_d = nc.dram_tensor("w1", [D, FF], BF16, kind="ExternalInput")
    w2_d = nc.dram_tensor("w2", [FF, D], BF16, kind="ExternalInput")
    bq_t_d = nc.dram_tensor("bq_t", [128, DC], F32, kind="ExternalInput")
    bk_t_d = nc.dram_tensor("bk_t", [128, DC], F32, kind="ExternalInput")
    bv_row_d = nc.dram_tensor("bv_row", [1, D], BF16, kind="ExternalInput")
    bo_row_d = nc.dram_tensor("bo_row", [1, D], BF16, kind="ExternalInput")
    b1_t_d = nc.dram_tensor("b1_t", [128, FC], F32, kind="ExternalInput")
    b2_row_d = nc.dram_tensor("b2_row", [1, D], BF16, kind="ExternalInput")
    out_d = nc.dram_tensor("out", [S, D], F32, kind="ExternalOutput")

    with TileContext(nc) as tc:
        # ---- pools (stack allocator: release order = reverse alloc order) ----
        persist = tc.alloc_tile_pool(name="persist", bufs=1)
        ps_a = tc.alloc_tile_pool(name="ps_a", bufs=2, space="PSUM")
        ps_b = tc.alloc_tile_pool(name="ps_b", bufs=2, space="PSUM")
        p_wst = tc.alloc_tile_pool(name="p_wst", bufs=4)  # streamed weight blocks
        p_ln = tc.alloc_tile_pool(name="p_ln", bufs=4)    # LN stats scratch
        p_src = tc.alloc_tile_pool(name="p_src", bufs=1)
        p_qkv = tc.alloc_tile_pool(name="p_qkv", bufs=1)
        p_ln1 = tc.alloc_tile_pool(name="p_ln1", bufs=1)

        # ---- persistent small tiles ----
        ident = persist.tile([128, 128], F32, tag="ident")
        masks.make_identity(nc, ident[:])
        eps_t = persist.tile([128, 1], F32, tag="eps")
        nc.vector.memset(eps_t[:], EPS)
        ones_bf = persist.tile([1, 128], BF16, tag="ones")
        nc.vector.memset(ones_bf[:], 1.0)
        bq_t = persist.tile([128, DC], F32, tag="bq")
        nc.sync.dma_start(bq_t[:], bq_t_d[:])
        bk_t = persist.tile([128, DC], F32, tag="bk")
        nc.sync.dma_start(bk_t[:], bk_t_d[:])
        bv_row = persist.tile([1, D], BF16, tag="bv")
        nc.sync.dma_start(bv_row[:], bv_row_d[:])
        bo_row = persist.tile([1, D], BF16, tag="bo")
        nc.sync.dma_start(bo_row[:], bo_row_d[:])
        b1_t = persist.tile([128, FC], F32, tag="b1")
        nc.sync.dma_start(b1_t[:], b1_t_d[:])
        b2_row = persist.tile([1, D], BF16, tag="b2")
        nc.sync.dma_start(b2_row[:], b2_row_d[:])

        def quad_psums(label):
            ps4 = [ps_a.tile([128, S], F32, tag="big", name=f"{label}a{i}")
                   for i in range(2)]
            ps4 += [ps_b.tile([128, S], F32, tag="ctx", name=f"{label}b{i}")
                    for i in range(2)]
            return ps4

        def layernorm(src_tiles, out_pool, out_tag):
            """f32 token-major tiles -> normalized f32 token-major tiles."""
            normed = []
            for i in range(SC):
                st = src_tiles[i]
                stats = p_ln.tile([128, 2, 6], F32, tag="stats",
                                  name=f"st_{out_tag}{i}")
                nc.vector.bn_stats(stats[:, 0, :], st[:, 0:512])
                nc.vector.bn_stats(stats[:, 1, :], st[:, 512:1024])
                mv = p_ln.tile([128, 2], F32, tag="mv", name=f"mv_{out_tag}{i}")
                nc.vector.bn_aggr(mv[:], stats[:])
                nc.scalar.activation(mv[:, 1:2], mv[:, 1:2], AF.Sqrt,
                                     bias=eps_t[:, 0:1])
                rsig = p_ln.tile([128, 1], F32, tag="rsig",
                                 name=f"rs_{out_tag}{i}")
                nc.vector.reciprocal_approx_fast(rsig[:], mv[:, 1:2])
                xt = out_pool.tile([128, D], F32, tag=f"{out_tag}{i}",
                                   name=f"x_{out_tag}{i}")
                nc.vector.tensor_scalar(
                    out=xt[:], in0=st[:], scalar1=mv[:, 0:1], scalar2=rsig[:],
                    op0=mybir.AluOpType.subtract, op1=mybir.AluOpType.mult)
                normed.append(xt)
            return normed

        def transpose_to_fm(x_tm_tiles, out_pool, out_tag):
            """f32 token-major [128,1024]x8 -> bf16 feature-major [128,1024]x8."""
            fm = []
            for j in range(DC):
                pt = ps_a.tile([128, S], F32, tag="big", name=f"pt_{out_tag}{j}")
                for i in range(SC):
                    nc.tensor.transpose(pt[:, i * 128:(i + 1) * 128],
                                        x_tm_tiles[i][:, j * 128:(j + 1) * 128],
                                        ident[:])
                ft = out_pool.tile([128, S], BF16, tag=f"{out_tag}{j}",
                                   name=f"f_{out_tag}{j}")
                nc.scalar.copy(ft[:], pt[:])
                fm.append(ft)
            return fm

        # ================= Phase A: load + LN1 + transpose =================
        src_t = []
        for i in range(SC):
            st = p_src.tile([128, D], F32, tag=f"src{i}", name=f"src{i}")
            nc.sync.dma_start(st[:], src_d[i * 128:(i + 1) * 128, :])
            src_t.append(st)

        x_tm = layernorm(src_t, p_ln1, "xtm")
        x_fm = transpose_to_fm(x_tm, p_ln1, "xfm")

        # ================= Phase B: QKV projections =================
        wv_sb = []
        for k in range(DC):
            wt = p_qkv.tile([128, D], BF16, tag=f"wv{k}", name=f"wv{k}")
            nc.sync.dma_start(wt[:], wv_d[k * 128:(k + 1) * 128, :])
            wv_sb.append(wt)

        qt_t, kt_t = [], []
        for nm, w_d, b_t, dst in (("q", wq_d, bq_t, qt_t), ("k", wk_d, bk_t, kt_t)):
            for mq in range(2):
                ps4 = quad_psums(f"{nm}{mq}")
                for k in range(DC):
                    wblk = p_wst.tile([128, 512], BF16, tag="wblk",
                                      name=f"w{nm}{mq}_{k}")
                    nc.sync.dma_start(
                        wblk[:], w_d[k * 128:(k + 1) * 128, mq * 512:(mq + 1) * 512])
                    for i in range(4):
                        _mm_halves(nc, ps4[i], wblk[:, i * 128:(i + 1) * 128],
                                   x_fm[k][:], start=(k == 0), stop=(k == DC - 1))
                for i in range(4):
                    m = mq * 4 + i
                    ot = p_qkv.tile([128, S], BF16, tag=f"{nm}t{m}",
                                    name=f"{nm}t{m}")
                    nc.scalar.activation(ot[:], ps4[i][:], AF.Identity,
                                         bias=b_t[:, m:m + 1])
                    dst.append(ot)

        # V token-major, interleaved [128, 16, 65] with ones column at 64
        v_aug = []
        for s in range(SC):
            ps = ps_a.tile([128, S], F32, tag="big", name=f"vps{s}")
            for k in range(DC):
                _mm_halves(nc, ps, x_fm[k][:, s * 128:(s + 1) * 128], wv_sb[k][:],
                           start=(k == 0), stop=(k == DC - 1 and bv_zero))
            if not bv_zero:
                for n0 in N2:
                    nc.tensor.matmul(ps[:, n0:n0 + 512], ones_bf[0:1, 0:128],
                                     bv_row[0:1, n0:n0 + 512], start=False,
                                     stop=True)
            va = p_qkv.tile([128, H, DH + 1], BF16, tag=f"va{s}", name=f"va{s}")
            nc.vector.memset(va[:, :, DH:DH + 1], 1.0)
            nc.scalar.copy(va[:, :, 0:DH],
                           ps[:].rearrange("p (h c) -> p h c", c=DH))
            v_aug.append(va)

        p_ln1.release()  # x_tm / x_fm no longer needed

        # ================= Phase C: attention (head-pipelined) =================
        p_wo = tc.alloc_tile_pool(name="p_wo", bufs=1)
        p_ctx = tc.alloc_tile_pool(name="p_ctx", bufs=1)
        p_attn = tc.alloc_tile_pool(name="p_attn", bufs=2)

        wo_sb = []
        for k in range(DC):
            wt = p_wo.tile([128, D], BF16, tag=f"wo{k}", name=f"wo{k}")
            nc.sync.dma_start(wt[:], wo_d[k * 128:(k + 1) * 128, :])
            wo_sb.append(wt)

        ctx_t = [p_ctx.tile([128, S], BF16, tag=f"ctx{t}", name=f"ctx{t}")
                 for t in range(DC)]

        prev = None  # (head, exp_tiles) from previous iteration
        for h in range(H + 1):
            if h < H:
                t, off = h // 2, (h % 2) * 64
                q_sl = qt_t[t][off:off + 64, :]
                exp_tiles = []
            psc = None
            if prev is not None:
                ph = prev[0]
                psc = ps_b.tile([DH + 1, S], F32, tag="ctx", name=f"psc{ph}")
            # ctx of head h-1 FIRST (its exps finished an iteration ago, so the
            # ctx matmuls and the normalize chain never wait on ACT's exp
            # queue), then scores+exp of head h.
            if prev is not None:
                ph, pexp = prev
                for sk in range(SC):
                    _mm_halves(nc, psc, v_aug[sk][:, ph, :], pexp[sk][:],
                               start=(sk == 0), stop=(sk == SC - 1))
                pt_, poff = ph // 2, (ph % 2) * 64
                den = p_attn.tile([1, S], F32, tag="den", name=f"den{ph}")
                nc.scalar.copy(den[:], psc[DH:DH + 1, :])
                rec = p_attn.tile([1, S], F32, tag="recip", name=f"rec{ph}")
                nc.vector.reciprocal_approx_fast(rec[:], den[:])
                bc = p_attn.tile([64, S], F32, tag="bcast", name=f"bc{ph}")
                nc.gpsimd.partition_broadcast(bc[:], rec[0:1, :])
                nc.vector.tensor_tensor(
                    out=ctx_t[pt_][poff:poff + 64, :], in0=psc[0:DH, :],
                    in1=bc[:], op=mybir.AluOpType.mult)
            if h < H:
                for sk in range(SC):
                    k_sl = kt_t[t][off:off + 64, sk * 128:(sk + 1) * 128]
                    pss = ps_a.tile([128, S], F32, tag="big", name=f"pss{h}_{sk}")
                    _mm_halves(nc, pss, k_sl, q_sl, start=True, stop=True)
                    et = p_attn.tile([128, S], BF16, tag=f"e{sk}",
                                     name=f"e{h}_{sk}")
                    nc.scalar.activation(et[:], pss[:], AF.Exp, scale=1.0 / 8.0)
                    exp_tiles.append(et)
            prev = (h, exp_tiles) if h < H else None

        # ================= Phase D: Wo projection + residual (in-place) ========
        for s in range(SC):
            ps = ps_a.tile([128, S], F32, tag="big", name=f"ops{s}")
            for k in range(DC):
                _mm_halves(nc, ps, ctx_t[k][:, s * 128:(s + 1) * 128], wo_sb[k][:],
                           start=(k == 0), stop=(k == DC - 1 and bo_zero))
            if not bo_zero:
                for n0 in N2:
                    nc.tensor.matmul(ps[:, n0:n0 + 512], ones_bf[0:1, 0:128],
                                     bo_row[0:1, n0:n0 + 512], start=False,
                                     stop=True)
            nc.vector.tensor_add(src_t[s][:], ps[:], src_t[s][:])
        src2_t = src_t

        p_attn.release()
        p_ctx.release()
        p_wo.release()
        p_qkv.release()

        # ================= Phase E: LN2 + transpose =================
        p_ffn = tc.alloc_tile_pool(name="p_ffn", bufs=1)
        p_out = tc.alloc_tile_pool(name="p_out", bufs=3)
        p_ytm = tc.alloc_tile_pool(name="p_ytm", bufs=1)

        y_tm = layernorm(src2_t, p_ytm, "ytm")
        y_fm = transpose_to_fm(y_tm, p_ffn, "yfm")
        p_ytm.release()

        # ================= Phase F: FFN =================
        mid_t = []
        for fq in range(FC // 4):
            ps4 = quad_psums(f"m{fq}")
            for k in range(DC):
                w1q = p_wst.tile([128, 512], BF16, tag="wblk", name=f"w1_{fq}_{k}")
                nc.sync.dma_start(
                    w1q[:], w1_d[k * 128:(k + 1) * 128, fq * 512:(fq + 1) * 512])
                for i in range(4):
                    _mm_halves(nc, ps4[i], w1q[:, i * 128:(i + 1) * 128],
                               y_fm[k][:], start=(k == 0), stop=(k == DC - 1))
            for i in range(4):
                f = fq * 4 + i
                mt = p_ffn.tile([128, S], BF16, tag=f"mid{f}", name=f"mid{f}")
                nc.scalar.activation(mt[:], ps4[i][:], AF.Relu,
                                     bias=b1_t[:, f:f + 1])
                mid_t.append(mt)

        for half in range(2):
            ps4 = quad_psums(f"o{half}")
            for k in range(FC):
                w2t = p_wst.tile([128, D], BF16, tag="w2s", name=f"w2_{half}_{k}")
                nc.sync.dma_start(w2t[:], w2_d[k * 128:(k + 1) * 128, :])
                for i in range(4):
                    s = half * 4 + i
                    _mm_halves(nc, ps4[i],
                               mid_t[k][:, s * 128:(s + 1) * 128], w2t[:],
                               start=(k == 0), stop=(k == FC - 1 and b2_zero))
            for i in range(4):
                s = half * 4 + i
                ps = ps4[i]
                if not b2_zero:
                    for n0 in N2:
                        nc.tensor.matmul(ps[:, n0:n0 + 512], ones_bf[0:1, 0:128],
                                         b2_row[0:1, n0:n0 + 512],
                                         start=False, stop=True,
                                         skip_group_check=True)
                ot = p_out.tile([128, D], F32, tag="outbuf", name=f"ob{s}")
                nc.vector.tensor_add(ot[:], ps[:], src2_t[s][:])
                nc.sync.dma_start(out_d[s * 128:(s + 1) * 128, :], ot[:])

        p_out.release()
        p_ffn.release()
        p_src.release()
        p_ln.release()
        p_wst.release()
        ps_b.release()
        ps_a.release()
        persist.release()

    nc.finalize()
    return nc


@lru_cache(maxsize=2)
def _get_nc(skip_bias):
    return build_nc(skip_bias)


def _prep_weights(inputs):
    bf = ml_dtypes.bfloat16
    f32 = np.float32
    g1 = np.asarray(inputs["g1"], f32)
    beta1 = np.asarray(inputs["beta1"], f32)
    g2 = np.asarray(inputs["g2"], f32)
    beta2 = np.asarray(inputs["beta2"], f32)
    Wq = np.asarray(inputs["Wq"], f32)
    Wk = np.asarray(inputs["Wk"], f32)
    Wv = np.asarray(inputs["Wv"], f32)
    Wo = np.asarray(inputs["Wo"], f32)
    W1 = np.asarray(inputs["W1"], f32)
    W2 = np.asarray(inputs["W2"], f32)

    # fold LN1 gamma/beta into QKV weights+biases, LN2 into W1/b1
    wq_eff = g1[:, None] * Wq
    wk_eff = g1[:, None] * Wk
    wv_eff = g1[:, None] * Wv
    bq_eff = np.asarray(inputs["bq"], f32) + beta1 @ Wq
    bk_eff = np.asarray(inputs["bk"], f32) + beta1 @ Wk
    bv_eff = np.asarray(inputs["bv"], f32) + beta1 @ Wv
    w1_eff = g2[:, None] * W1
    b1_eff = np.asarray(inputs["b1"], f32) + beta2 @ W1
    b2_eff = np.asarray(inputs["b2"], f32)
    bo_eff = np.asarray(inputs["bo"], f32)

    weights = {
        "wq": np.ascontiguousarray(wq_eff.astype(bf)),
        "wk": np.ascontiguousarray(wk_eff.astype(bf)),
        "wv": np.ascontiguousarray(wv_eff.astype(bf)),
        "wo": np.ascontiguousarray(Wo.astype(bf)),
        "w1": np.ascontiguousarray(w1_eff.astype(bf)),
        "w2": np.ascontiguousarray(W2.astype(bf)),
        "bq_t": np.ascontiguousarray(bq_eff.reshape(DC, 128).T.astype(f32)),
        "bk_t": np.ascontiguousarray(bk_eff.reshape(DC, 128).T.astype(f32)),
        "bv_row": np.ascontiguousarray(bv_eff.reshape(1, D).astype(bf)),
        "bo_row": np.ascontiguousarray(bo_eff.reshape(1, D).astype(bf)),
        "b1_t": np.ascontiguousarray(b1_eff.reshape(FC, 128).T.astype(f32)),
        "b2_row": np.ascontiguousarray(b2_eff.reshape(1, D).astype(bf)),
    }
    skip_bias = (bool(np.all(bv_eff == 0)), bool(np.all(bo_eff == 0)),
                 bool(np.all(b2_eff == 0)))
    return weights, skip_bias


def run_kernel(inputs, trace=False):
    weights, skip_bias = _prep_weights(inputs)
    nc = _get_nc(skip_bias)
    src = np.asarray(inputs["src"], np.float32)
    in_maps = [dict(weights, src=np.ascontiguousarray(src[i])) for i in range(B)]
    res = run_bass_kernel_spmd(nc, in_maps, list(range(B)), trace=trace)
    out = np.stack([res.results[i]["out"] for i in range(B)], axis=0)
    return out, res


def kernel(**inputs) -> np.ndarray:
    out, _ = run_kernel(inputs, trace=False)
    return out


# revision 17
# speedup vs baseline: 1.2035x; 1.0345x over previous
"""Trainium2 Bass kernel for nn_CustomEncoderLayer (dense transformer encoder layer).

Sharding: pure data-parallel over batch — 8 batch elements -> 8 NeuronCores,
each core runs the full encoder layer on its [1024, 1024] slice. Weights are
replicated to every core; no collectives.

Per-core pipeline (S=1024 tokens, D=1024, H=16 heads, Dh=64, F=4096):
  A. load src (token-major, f32), LayerNorm1 (gamma/beta folded into W on host),
     PE-transpose x -> feature-major bf16
  B. Q^T,K^T projections (feature-major, bf16, quad-blocked weight streaming),
     V (token-major, interleaved with a ones-column per head that accumulates
     the softmax denominator during the ctx matmul)
  C. attention, software-pipelined across heads: head h's scoresT+exp overlap
     head h-1's ctx matmuls. scoresT = k_h^T q_h (PSUM f32) -> exp (ACT,
     scale=1/8; no max subtraction — |scores| < ~3 so exp is safe in f32) ->
     ctx_augT = [v_h|1]^T exp^T (row 64 = denominator) -> normalize via
     reciprocal_approx_fast + gpsimd partition-broadcast + DVE multiply
  D. attn_out = ctx^T-stationary matmul with Wo, + residual (f32, in-place)
  E. LayerNorm2, transpose -> y feature-major bf16
  F. FFN: mid = relu(y@W1+b1) (bf16, 8MB resident, quad-blocked W1 streaming),
     out = mid@W2 + b2 + residual (W2 streamed in two s-halves)

All matmul operands are bf16 (fp32 PSUM accumulation). LayerNorm stats and
residuals stay fp32. Expected rel err vs fp32 reference ~1e-3.
"""

from functools import lru_cache

import numpy as np
import ml_dtypes

import concourse.bacc as bacc
import concourse.mybir as mybir
from concourse import masks
from concourse.tile import TileContext
from concourse.bass_utils import run_bass_kernel_spmd

F32 = mybir.dt.float32
BF16 = mybir.dt.bfloat16
AF = mybir.ActivationFunctionType

B, S, D = 8, 1024, 1024
H, DH = 16, 64
FF = 4096
EPS = 1e-5
SC = S // 128   # 8 token chunks
DC = D // 128   # 8 feature chunks
FC = FF // 128  # 32 ff chunks
N2 = (0, 512)   # free-dim halves


def _mm_halves(nc, ps, lhsT, rhs_full, start, stop):
    """Two N=512 matmuls covering a [*, 1024] psum tile."""
    for n0 in N2:
        nc.tensor.matmul(ps[:, n0:n0 + 512], lhsT, rhs_full[:, n0:n0 + 512],
                         start=start, stop=stop)


def build_nc(skip_bias=(False, False, False)):
    """skip_bias = (bv_zero, bo_zero, b2_zero): skip K=1 bias matmuls."""
    bv_zero, bo_zero, b2_zero = skip_bias
    nc = bacc.Bacc("TRN2", target_bir_lowering=False, num_devices=8)

    src_d = nc.dram_tensor("src", [S, D], F32, kind="ExternalInput")
    wq_d = nc.dram_tensor("wq", [D, D], BF16, kind="ExternalInput")
    wk_d = nc.dram_tensor("wk", [D, D], BF16, kind="ExternalInput")
    wv_d = nc.dram_tensor("wv", [D, D], BF16, kind="ExternalInput")
    wo_d = nc.dram_tensor("wo", [D, D], BF16, kind="ExternalInput")
    w1_d = nc.dram_tensor("w1", [D, FF], BF16, kind="ExternalInput")
    w2_d = nc.dram_tensor("w2", [FF, D], BF16, kind="ExternalInput")
    bq_t_d = nc.dram_tensor("bq_t", [128, DC], F32, kind="ExternalInput")
    bk_t_d = nc.dram_tensor("bk_t", [128, DC], F32, kind="ExternalInput")
    bv_row_d = nc.dram_tensor("bv_row", [1, D], BF16, kind="ExternalInput")
    bo_row_d = nc.dram_tensor("bo_row", [1, D], BF16, kind="ExternalInput")
    b1_t_d = nc.dram_tensor("b1_t", [128, FC], F32, kind="ExternalInput")
    b2_row_d = nc.dram_tensor("b2_row", [1, D], BF16, kind="ExternalInput")
    out_d = nc.dram_tensor("out", [S, D], F32, kind="ExternalOutput")

    with TileContext(nc) as tc:
        # ---- pools (stack allocator: release order = reverse alloc order) ----
        persist = tc.alloc_tile_pool(name="persist", bufs=1)
        ps_a = tc.alloc_tile_pool(name="ps_a", bufs=2, space="PSUM")
        ps_b = tc.alloc_tile_pool(name="ps_b", bufs=2, space="PSUM")
        p_wst = tc.alloc_tile_pool(name="p_wst", bufs=6)  # streamed weight blocks
        p_ln = tc.alloc_tile_pool(name="p_ln", bufs=4)    # LN stats scratch
        p_src = tc.alloc_tile_pool(name="p_src", bufs=1)
        p_qkv = tc.alloc_tile_pool(name="p_qkv", bufs=1)
        p_ln1 = tc.alloc_tile_pool(name="p_ln1", bufs=1)

        # ---- persistent small tiles ----
        ident = persist.tile([128, 128], F32, tag="ident")
        masks.make_identity(nc, ident[:])
        eps_t = persist.tile([128, 1], F32, tag="eps")
        nc.vector.memset(eps_t[:], EPS)
        ones_bf = persist.tile([1, 128], BF16, tag="ones")
        nc.vector.memset(ones_bf[:], 1.0)
        bq_t = persist.tile([128, DC], F32, tag="bq")
        nc.sync.dma_start(bq_t[:], bq_t_d[:])
        bk_t = persist.tile([128, DC], F32, tag="bk")
        nc.sync.dma_start(bk_t[:], bk_t_d[:])
        bv_row = persist.tile([1, D], BF16, tag="bv")
        nc.sync.dma_start(bv_row[:], bv_row_d[:])
        bo_row = persist.tile([1, D], BF16, tag="bo")
        nc.sync.dma_start(bo_row[:], bo_row_d[:])
        b1_t = persist.tile([128, FC], F32, tag="b1")
        nc.sync.dma_start(b1_t[:], b1_t_d[:])
        b2_row = persist.tile([1, D], BF16, tag="b2")
        nc.sync.dma_start(b2_row[:], b2_row_d[:])

        def quad_psums(label):
            ps4 = [ps_a.tile([128, S], F32, tag="big", name=f"{label}a{i}")
                   for i in range(2)]
            ps4 += [ps_b.tile([128, S], F32, tag="ctx", name=f"{label}b{i}")
                    for i in range(2)]
            return ps4

        def layernorm(src_tiles, out_pool, out_tag):
            """f32 token-major tiles -> normalized f32 token-major tiles."""
            normed = []
            for i in range(SC):
                st = src_tiles[i]
                stats = p_ln.tile([128, 2, 6], F32, tag="stats",
                                  name=f"st_{out_tag}{i}")
                nc.vector.bn_stats(stats[:, 0, :], st[:, 0:512])
                nc.vector.bn_stats(stats[:, 1, :], st[:, 512:1024])
                mv = p_ln.tile([128, 2], F32, tag="mv", name=f"mv_{out_tag}{i}")
                nc.vector.bn_aggr(mv[:], stats[:])
                nc.scalar.activation(mv[:, 1:2], mv[:, 1:2], AF.Sqrt,
                                     bias=eps_t[:, 0:1])
                rsig = p_ln.tile([128, 1], F32, tag="rsig",
                                 name=f"rs_{out_tag}{i}")
                nc.vector.reciprocal_approx_fast(rsig[:], mv[:, 1:2])
                xt = out_pool.tile([128, D], F32, tag=f"{out_tag}{i}",
                                   name=f"x_{out_tag}{i}")
                nc.vector.tensor_scalar(
                    out=xt[:], in0=st[:], scalar1=mv[:, 0:1], scalar2=rsig[:],
                    op0=mybir.AluOpType.subtract, op1=mybir.AluOpType.mult)
                normed.append(xt)
            return normed

        def transpose_to_fm(x_tm_tiles, out_pool, out_tag):
            """f32 token-major [128,1024]x8 -> bf16 feature-major [128,1024]x8."""
            fm = []
            for j in range(DC):
                pt = ps_a.tile([128, S], F32, tag="big", name=f"pt_{out_tag}{j}")
                for i in range(SC):
                    nc.tensor.transpose(pt[:, i * 128:(i + 1) * 128],
                                        x_tm_tiles[i][:, j * 128:(j + 1) * 128],
                                        ident[:])
                ft = out_pool.tile([128, S], BF16, tag=f"{out_tag}{j}",
                                   name=f"f_{out_tag}{j}")
                nc.scalar.copy(ft[:], pt[:])
                fm.append(ft)
            return fm

        # ================= Phase A: load + LN1 + transpose =================
        src_t = []
        for i in range(SC):
            st = p_src.tile([128, D], F32, tag=f"src{i}", name=f"src{i}")
            nc.sync.dma_start(st[:], src_d[i * 128:(i + 1) * 128, :])
            src_t.append(st)

        x_tm = layernorm(src_t, p_ln1, "xtm")
        x_fm = transpose_to_fm(x_tm, p_ln1, "xfm")

        # ================= Phase B: QKV projections =================
        wv_sb = []
        for k in range(DC):
            wt = p_qkv.tile([128, D], BF16, tag=f"wv{k}", name=f"wv{k}")
            nc.sync.dma_start(wt[:], wv_d[k * 128:(k + 1) * 128, :])
            wv_sb.append(wt)

        qt_t, kt_t = [], []
        for nm, w_d, b_t, dst in (("q", wq_d, bq_t, qt_t), ("k", wk_d, bk_t, kt_t)):
            for mq in range(2):
                ps4 = quad_psums(f"{nm}{mq}")
                for k in range(DC):
                    wblk = p_wst.tile([128, 512], BF16, tag="wblk",
                                      name=f"w{nm}{mq}_{k}")
                    nc.sync.dma_start(
                        wblk[:], w_d[k * 128:(k + 1) * 128, mq * 512:(mq + 1) * 512])
                    for i in range(4):
                        _mm_halves(nc, ps4[i], wblk[:, i * 128:(i + 1) * 128],
                                   x_fm[k][:], start=(k == 0), stop=(k == DC - 1))
                for i in range(4):
                    m = mq * 4 + i
                    ot = p_qkv.tile([128, S], BF16, tag=f"{nm}t{m}",
                                    name=f"{nm}t{m}")
                    nc.scalar.activation(ot[:], ps4[i][:], AF.Identity,
                                         bias=b_t[:, m:m + 1])
                    dst.append(ot)

        # V token-major, interleaved [128, 16, 65] with ones column at 64
        v_aug = []
        for s in range(SC):
            ps = ps_a.tile([128, S], F32, tag="big", name=f"vps{s}")
            for k in range(DC):
                _mm_halves(nc, ps, x_fm[k][:, s * 128:(s + 1) * 128], wv_sb[k][:],
                           start=(k == 0), stop=(k == DC - 1 and bv_zero))
            if not bv_zero:
                for n0 in N2:
                    nc.tensor.matmul(ps[:, n0:n0 + 512], ones_bf[0:1, 0:128],
                                     bv_row[0:1, n0:n0 + 512], start=False,
                                     stop=True)
            va = p_qkv.tile([128, H, DH + 1], BF16, tag=f"va{s}", name=f"va{s}")
            nc.vector.memset(va[:, :, DH:DH + 1], 1.0)
            nc.scalar.copy(va[:, :, 0:DH],
                           ps[:].rearrange("p (h c) -> p h c", c=DH))
            v_aug.append(va)

        p_ln1.release()  # x_tm / x_fm no longer needed

        # ================= Phase C: attention (head-pipelined) =================
        p_wo = tc.alloc_tile_pool(name="p_wo", bufs=1)
        p_ctx = tc.alloc_tile_pool(name="p_ctx", bufs=1)
        p_attn = tc.alloc_tile_pool(name="p_attn", bufs=2)

        wo_sb = []
        for k in range(DC):
            wt = p_wo.tile([128, D], BF16, tag=f"wo{k}", name=f"wo{k}")
            nc.sync.dma_start(wt[:], wo_d[k * 128:(k + 1) * 128, :])
            wo_sb.append(wt)

        ctx_t = [p_ctx.tile([128, S], BF16, tag=f"ctx{t}", name=f"ctx{t}")
                 for t in range(DC)]

        prev = None  # (head, exp_tiles) from previous iteration
        for h in range(H + 1):
            if h < H:
                t, off = h // 2, (h % 2) * 64
                q_sl = qt_t[t][off:off + 64, :]
                exp_tiles = []
            psc = None
            if prev is not None:
                ph = prev[0]
                psc = ps_b.tile([DH + 1, S], F32, tag="ctx", name=f"psc{ph}")
            # ctx of head h-1 FIRST (its exps finished an iteration ago, so the
            # ctx matmuls and the normalize chain never wait on ACT's exp
            # queue), then scores+exp of head h.
            if prev is not None:
                ph, pexp = prev
                for sk in range(SC):
                    _mm_halves(nc, psc, v_aug[sk][:, ph, :], pexp[sk][:],
                               start=(sk == 0), stop=(sk == SC - 1))
                pt_, poff = ph // 2, (ph % 2) * 64
                den = p_attn.tile([1, S], F32, tag="den", name=f"den{ph}")
                nc.scalar.copy(den[:], psc[DH:DH + 1, :])
                rec = p_attn.tile([1, S], F32, tag="recip", name=f"rec{ph}")
                nc.vector.reciprocal_approx_fast(rec[:], den[:])
                bc = p_attn.tile([64, S], F32, tag="bcast", name=f"bc{ph}")
                nc.gpsimd.partition_broadcast(bc[:], rec[0:1, :])
                nc.vector.tensor_tensor(
                    out=ctx_t[pt_][poff:poff + 64, :], in0=psc[0:DH, :],
                    in1=bc[:], op=mybir.AluOpType.mult)
            if h < H:
                for sk in range(SC):
                    k_sl = kt_t[t][off:off + 64, sk * 128:(sk + 1) * 128]
                    pss = ps_a.tile([128, S], F32, tag="big", name=f"pss{h}_{sk}")
                    _mm_halves(nc, pss, k_sl, q_sl, start=True, stop=True)
                    et = p_attn.tile([128, S], BF16, tag=f"e{sk}",
                                     name=f"e{h}_{sk}")
                    nc.scalar.activation(et[:], pss[:], AF.Exp, scale=1.0 / 8.0)
                    exp_tiles.append(et)
            prev = (h, exp_tiles) if h < H else None

        # ================= Phase D: Wo projection + residual (in-place) ========
        for s in range(SC):
            ps = ps_a.tile([128, S], F32, tag="big", name=f"ops{s}")
            for k in range(DC):
                _mm_halves(nc, ps, ctx_t[k][:, s * 128:(s + 1) * 128], wo_sb[k][:],
                           start=(k == 0), stop=(k == DC - 1 and bo_zero))
            if not bo_zero:
                for n0 in N2:
                    nc.tensor.matmul(ps[:, n0:n0 + 512], ones_bf[0:1, 0:128],
                                     bo_row[0:1, n0:n0 + 512], start=False,
                                     stop=True)
            nc.vector.tensor_add(src_t[s][:], ps[:], src_t[s][:])
        src2_t = src_t

        p_attn.release()
        p_ctx.release()
        p_wo.release()
        p_qkv.release()

        # ================= Phase E: LN2 + transpose =================
        p_ffn = tc.alloc_tile_pool(name="p_ffn", bufs=1)
        p_out = tc.alloc_tile_pool(name="p_out", bufs=3)
        p_ytm = tc.alloc_tile_pool(name="p_ytm", bufs=1)

        y_tm = layernorm(src2_t, p_ytm, "ytm")
        y_fm = transpose_to_fm(y_tm, p_ffn, "yfm")
        p_ytm.release()
        p_w2s = tc.alloc_tile_pool(name="p_w2s", bufs=6)

        # ================= Phase F: FFN =================
        mid_t = []
        for fq in range(FC // 4):
            ps4 = quad_psums(f"m{fq}")
            for k in range(DC):
                w1q = p_wst.tile([128, 512], BF16, tag="wblk", name=f"w1_{fq}_{k}")
                nc.sync.dma_start(
                    w1q[:], w1_d[k * 128:(k + 1) * 128, fq * 512:(fq + 1) * 512])
                for i in range(4):
                    _mm_halves(nc, ps4[i], w1q[:, i * 128:(i + 1) * 128],
                               y_fm[k][:], start=(k == 0), stop=(k == DC - 1))
            for i in range(4):
                f = fq * 4 + i
                mt = p_ffn.tile([128, S], BF16, tag=f"mid{f}", name=f"mid{f}")
                nc.scalar.activation(mt[:], ps4[i][:], AF.Relu,
                                     bias=b1_t[:, f:f + 1])
                mid_t.append(mt)

        for half in range(2):
            ps4 = quad_psums(f"o{half}")
            for k in range(FC):
                w2t = p_w2s.tile([128, D], BF16, tag="w2s", name=f"w2_{half}_{k}")
                nc.sync.dma_start(w2t[:], w2_d[k * 128:(k + 1) * 128, :])
                for i in range(4):
                    s = half * 4 + i
                    _mm_halves(nc, ps4[i],
                               mid_t[k][:, s * 128:(s + 1) * 128], w2t[:],
                               start=(k == 0), stop=(k == FC - 1 and b2_zero))
            for i in range(4):
                s = half * 4 + i
                ps = ps4[i]
                if not b2_zero:
                    for n0 in N2:
                        nc.tensor.matmul(ps[:, n0:n0 + 512], ones_bf[0:1, 0:128],
                                         b2_row[0:1, n0:n0 + 512],
                                         start=False, stop=True,
                                         skip_group_check=True)
                ot = p_out.tile([128, D], F32, tag="outbuf", name=f"ob{s}")
                nc.vector.tensor_add(ot[:], ps[:], src2_t[s][:])
                nc.sync.dma_start(out_d[s * 128:(s + 1) * 128, :], ot[:])

        p_w2s.release()
        p_out.release()
        p_ffn.release()
        p_src.release()
        p_ln.release()
        p_wst.release()
        ps_b.release()
        ps_a.release()
        persist.release()

    nc.finalize()
    return nc


@lru_cache(maxsize=2)
def _get_nc(skip_bias):
    return build_nc(skip_bias)


def _prep_weights(inputs):
    bf = ml_dtypes.bfloat16
    f32 = np.float32
    g1 = np.asarray(inputs["g1"], f32)
    beta1 = np.asarray(inputs["beta1"], f32)
    g2 = np.asarray(inputs["g2"], f32)
    beta2 = np.asarray(inputs["beta2"], f32)
    Wq = np.asarray(inputs["Wq"], f32)
    Wk = np.asarray(inputs["Wk"], f32)
    Wv = np.asarray(inputs["Wv"], f32)
    Wo = np.asarray(inputs["Wo"], f32)
    W1 = np.asarray(inputs["W1"], f32)
    W2 = np.asarray(inputs["W2"], f32)

    # fold LN1 gamma/beta into QKV weights+biases, LN2 into W1/b1
    wq_eff = g1[:, None] * Wq
    wk_eff = g1[:, None] * Wk
    wv_eff = g1[:, None] * Wv
    bq_eff = np.asarray(inputs["bq"], f32) + beta1 @ Wq
    bk_eff = np.asarray(inputs["bk"], f32) + beta1 @ Wk
    bv_eff = np.asarray(inputs["bv"], f32) + beta1 @ Wv
    w1_eff = g2[:, None] * W1
    b1_eff = np.asarray(inputs["b1"], f32) + beta2 @ W1
    b2_eff = np.asarray(inputs["b2"], f32)
    bo_eff = np.asarray(inputs["bo"], f32)

    weights = {
        "wq": np.ascontiguousarray(wq_eff.astype(bf)),
        "wk": np.ascontiguousarray(wk_eff.astype(bf)),
        "wv": np.ascontiguousarray(wv_eff.astype(bf)),
        "wo": np.ascontiguousarray(Wo.astype(bf)),
        "w1": np.ascontiguousarray(w1_eff.astype(bf)),
        "w2": np.ascontiguousarray(W2.astype(bf)),
        "bq_t": np.ascontiguousarray(bq_eff.reshape(DC, 128).T.astype(f32)),
        "bk_t": np.ascontiguousarray(bk_eff.reshape(DC, 128).T.astype(f32)),
        "bv_row": np.ascontiguousarray(bv_eff.reshape(1, D).astype(bf)),
        "bo_row": np.ascontiguousarray(bo_eff.reshape(1, D).astype(bf)),
        "b1_t": np.ascontiguousarray(b1_eff.reshape(FC, 128).T.astype(f32)),
        "b2_row": np.ascontiguousarray(b2_eff.reshape(1, D).astype(bf)),
    }
    skip_bias = (bool(np.all(bv_eff == 0)), bool(np.all(bo_eff == 0)),
                 bool(np.all(b2_eff == 0)))
    return weights, skip_bias


def run_kernel(inputs, trace=False):
    weights, skip_bias = _prep_weights(inputs)
    nc = _get_nc(skip_bias)
    src = np.asarray(inputs["src"], np.float32)
    in_maps = [dict(weights, src=np.ascontiguousarray(src[i])) for i in range(B)]
    res = run_bass_kernel_spmd(nc, in_maps, list(range(B)), trace=trace)
    out = np.stack([res.results[i]["out"] for i in range(B)], axis=0)
    return out, res


def kernel(**inputs) -> np.ndarray:
    out, _ = run_kernel(inputs, trace=False)
    return out
